# revision 25
# baseline (speedup 1.0000x reference)
"""Trainium2 Bass kernel for nn_DecoderLayer (dense transformer decoder layer).

Sharding: data-parallel over batch (16 batches -> 8 cores x 2 each). Each core
runs the full decoder layer on its batch slice; no collectives.

Layout: activations are kept feature-major ("xT": [feature partitions, token
free]) so every linear is psum = W^T.T @ xT on the PE with fp32r inputs
(1 cycle/row at N>=256). Weights stream from DRAM in natural [O, I] layout and
are transposed on-chip with PE-transpose blocks. Attention uses transposed
scores S^T = K_hT.T @ Q_hT ([j partitions, i free]), exp without
max-subtraction (scores are bounded, |s|*scale < ~2), and a ones-column
appended to V so the softmax denominator comes out of the same PV matmul.
LayerNorm runs feature-major with partition sums via ones-vector matmuls and
per-token broadcast via K=1 matmuls.

fp8 fast path: the cross-attention K/V projections (the dominant GEMMs, over
S=4096 enc tokens) and all PV matmuls run in fp8e4 with the PE's DoubleRow
mode (two K-blocks contracted per instruction at 0.5 cycles/row = 4x fp32r).
Quantization noise in K, V, and the exp'd scores sits entirely behind the
softmax average over 4096 (resp. 256) positions, so it attenuates by ~64x
(resp. 16x) before reaching the residual stream. The K-projection weights are
pre-scaled by 16 (folded into the PE transpose via a scaled identity) to
dodge fp8 subnormals, and the exp scale divides it back out. The V path's
16x rides through PV and is cancelled by a 16.0 ones-column (the softmax
denominator picks up the same factor). Scores themselves stay fp32r.

Cross-attention runs in jc-groups of 4 chunks whose PV partials accumulate
in PSUM across the group, quartering the DVE accumulate traffic.
"""
import sys
import numpy as np

sys.path.insert(0, '/opt/trn_rl_repo')

import concourse.bass as bass  # noqa: E402
import concourse.tile as tile  # noqa: E402
from concourse import bacc, mybir  # noqa: E402
from concourse.bass_utils import run_bass_kernel_spmd  # noqa: E402
from concourse.masks import make_identity  # noqa: E402
from contextlib import ExitStack  # noqa: E402

F32 = mybir.dt.float32
F32R = mybir.dt.float32r
F8 = mybir.dt.float8e4
BF16 = mybir.dt.bfloat16
AF = mybir.ActivationFunctionType
DR = mybir.MatmulPerfMode.DoubleRow

EPS = 1e-5
N_CORES = 8
W8SCALE = 16.0


def build_decoder(nc, tc, ctx, B_loc, NQ, S, W, NH, MLP, JC=256, suffix=""):
    HD = W // NH
    assert HD == 64 and NQ % 128 == 0 and W % 512 == 0 and JC % 128 == 0
    T = B_loc * NQ          # decoder tokens per core
    TC = T // 128
    WC = W // 128
    MC = MLP // 128
    NJC = S // JC           # enc chunks per batch
    JSC = JC // 128
    NQC = NQ // 128
    SCALE = float(W) ** -0.5
    HPC = 128 // HD         # heads per feature chunk (2)
    G = 4                   # jc chunks per PV-accumulation group
    assert NJC % G == 0 and JSC == 2 and NQC == 2

    dram = {}
    for name, shape in [
        ('query', [B_loc, NQ, W]), ('enc_mem', [B_loc, S, W]),
        ('out_pos_enc', [B_loc, NQ, W]),
        ('sa_wq', [W, W]), ('sa_wk', [W, W]), ('sa_wv', [W, W]), ('sa_wo', [W, W]),
        ('ca_wq', [W, W]), ('ca_wk', [W, W]), ('ca_wv', [W, W]), ('ca_wo', [W, W]),
        ('ffn_w1', [MLP, W]), ('ffn_b1', [MLP]), ('ffn_w2', [W, MLP]), ('ffn_b2', [W]),
        ('ln1_g', [W]), ('ln1_b', [W]), ('ln2_g', [W]), ('ln2_b', [W]),
        ('ln3_g', [W]), ('ln3_b', [W]),
    ]:
        if suffix:
            dram[name] = build_decoder._dram_cache[name]
        else:
            dram[name] = nc.dram_tensor(name, shape, F32, kind="ExternalInput")
    build_decoder._dram_cache = dict(dram)
    out_d = nc.dram_tensor("out" + suffix, [B_loc, NQ, W], F32,
                           kind="ExternalOutput")

    q_flat = dram['query'].rearrange("b n w -> (b n) w")
    pe_flat = dram['out_pos_enc'].rearrange("b n w -> (b n) w")
    m_flat = dram['enc_mem'].rearrange("b s w -> (b s) w")
    out_flat = out_d.rearrange("b n w -> (b n) w")

    # ---------------- global pools ----------------
    consts = ctx.enter_context(tc.tile_pool(name="consts", bufs=1))
    persist = ctx.enter_context(tc.tile_pool(name="persist", bufs=1))
    scratch = ctx.enter_context(tc.tile_pool(name="scratch", bufs=2))
    tp_ps = ctx.enter_context(tc.tile_pool(name="tp_ps", bufs=2, space="PSUM"))
    mm_ps = ctx.enter_context(tc.tile_pool(name="mm_ps", bufs=2, space="PSUM"))
    sc_ps = ctx.enter_context(tc.tile_pool(name="sc_ps", bufs=3, space="PSUM"))
    pv_ps = ctx.enter_context(tc.tile_pool(name="pv_ps", bufs=1, space="PSUM"))

    ident = consts.tile([128, 128], F32, tag="ident")
    make_identity(nc, ident[:])
    ones_f = consts.tile([128, 128], F32, tag="ones_f")
    nc.gpsimd.memset(ones_f[:], 1.0)
    ones_r = consts.tile([128, 128], F32R, tag="ones_r")
    nc.vector.tensor_copy(ones_r[:], ones_f[:])
    ident_r = consts.tile([128, 128], F32R, tag="ident_r")
    nc.vector.tensor_copy(ident_r[:], ident[:])
    ident16 = consts.tile([128, 128], F32R, tag="ident16")
    nc.vector.tensor_scalar_mul(ident16[:], ident[:], W8SCALE)
    eps_t = consts.tile([1, 1], F32, tag="eps")
    nc.gpsimd.memset(eps_t[:], EPS)

    def load_col(name, n):
        """[n] param vector -> [128, n/128] per-partition columns."""
        nch = n // 128
        land = scratch.tile([128, 128], F32, tag="colland", bufs=2,
                            name=name + "_land")
        nc.sync.dma_start(land[0:nch, :],
                          dram[name].rearrange("(c p) -> c p", p=128))
        pt = tp_ps.tile([128, 512], F32, tag="tp", name="pt_col")
        nc.tensor.transpose(pt[:, 0:128], land[:, 0:128], ident[:])
        t = consts.tile([128, nch], F32, tag=name, name=name + "_col")
        nc.vector.tensor_copy(t[:], pt[:, 0:nch])
        return t
    cols = {k: load_col(k, W) for k in
            ['ln1_g', 'ln1_b', 'ln2_g', 'ln2_b', 'ln3_g', 'ln3_b', 'ffn_b2']}
    b1_col = load_col('ffn_b1', MLP)

    # ---------------- helpers ----------------
    def evict(engine, dst, src):
        if engine == 'act':
            nc.scalar.activation(dst, src, AF.Copy)
        else:
            nc.vector.tensor_copy(dst, src)

    def transpose_group(dst_slice, src_slices, exact=False, scale16=False,
                        ev='dve', n2=128):
        """Transpose up to 4 [128,128] blocks through one PSUM bank, evict
        with a single copy on `ev` engine. fp32r by default; scale16
        multiplies the transpose by W8SCALE (for fp8 weight tiles)."""
        n = len(src_slices)
        dt = F32 if exact else F32R
        idn = ident if exact else (ident16 if scale16 else ident_r)
        pt = tp_ps.tile([128, 512], dt, tag="tp", name="ptg")
        for i, src in enumerate(src_slices):
            s = src if exact else src.bitcast(F32R)
            nc.tensor.transpose(pt[:, i * 128:(i + 1) * 128], s, idn[:])
        src_view = pt[:, 0:n * 128]
        if len(dst_slice.shape) == 3:
            src_view = src_view.rearrange("p (c n2) -> p c n2", n2=n2)
        evict(ev, dst_slice, src_view)

    def load_wT(pool, tag, w_dram, O, I, name, wr_pool=None, dtype=F32R,
                scale16=False, ev='dve', bufs=1):
        """Stream W [O, I] from DRAM -> W^T tile [128, I/128, O] (dtype)."""
        wt = pool.tile([128, I // 128, O], dtype, tag=tag, name=name,
                       bufs=bufs)
        for ob in range(O // 128):
            if wr_pool is not None:
                wr = wr_pool.tile([128, I], F32R, tag="wrow_big", bufs=4,
                                  name="wrb")
                nc.sync.dma_start(
                    wr[:], w_dram[ob * 128:(ob + 1) * 128, :].bitcast(F32R))
                for half in range(I // 512):
                    transpose_group(
                        wt[:, half * 4:(half + 1) * 4,
                           ob * 128:(ob + 1) * 128],
                        [wr[:, half * 512 + k * 128:half * 512 + (k + 1) * 128]
                         for k in range(4)], scale16=scale16, ev=ev)
            else:
                for half in range(I // 512):
                    wr = scratch.tile([128, 512], F32R, tag="wrow", bufs=5,
                                      name="wr")
                    nc.sync.dma_start(
                        wr[:], w_dram[ob * 128:(ob + 1) * 128,
                                      half * 512:(half + 1) * 512]
                        .bitcast(F32R))
                    transpose_group(
                        wt[:, half * 4:(half + 1) * 4,
                           ob * 128:(ob + 1) * 128],
                        [wr[:, k * 128:(k + 1) * 128] for k in range(4)],
                        scale16=scale16, ev=ev)
        return wt

    def gemm(psum, wt, oc, rhs_fn, ICn):
        for ic in range(ICn):
            nc.tensor.matmul(psum, wt[:, ic, oc * 128:(oc + 1) * 128],
                             rhs_fn(ic), start=(ic == 0), stop=(ic == ICn - 1))

    def layernorm(x_fn, n_chunks, N, g_col, b_col, out_fn, xm_pool=True):
        """Feature-major LN over the partition (feature) dim."""
        ps_s = tp_ps.tile([1, N], F32, tag="tp", name="ps_s")
        for ic in range(n_chunks):
            nc.tensor.matmul(ps_s[0:1, :], ones_r[:, 0:1], x_fn(ic),
                             start=(ic == 0), stop=(ic == n_chunks - 1))
        ps_q = tp_ps.tile([1, N], F32, tag="tp", name="ps_q")
        for ic in range(n_chunks):
            sq = scratch.tile([128, N], F32R, tag="sq", name="sq")
            nc.gpsimd.tensor_mul(sq[:, 0:N], x_fn(ic), x_fn(ic))
            nc.tensor.matmul(ps_q[0:1, :], ones_r[:, 0:1], sq[:, 0:N],
                             start=(ic == 0), stop=(ic == n_chunks - 1))
        inv_w = 1.0 / (n_chunks * 128)
        mu = scratch.tile([1, N], F32R, tag="st_mu", bufs=1, name="mu")
        nc.scalar.activation(mu[0:1, :], ps_s[0:1, :], AF.Copy, scale=inv_w)
        ex2 = scratch.tile([1, N], F32, tag="st_e", bufs=1, name="ex2")
        nc.scalar.activation(ex2[0:1, :], ps_q[0:1, :], AF.Copy, scale=inv_w)
        mu2 = scratch.tile([1, N], F32, tag="st_x", bufs=1, name="mu2")
        nc.vector.tensor_mul(mu2[0:1, :], mu[0:1, :], mu[0:1, :])
        var = scratch.tile([1, N], F32, tag="st_v", bufs=1, name="var")
        nc.vector.tensor_sub(var[0:1, :], ex2[0:1, :], mu2[0:1, :])
        sd = scratch.tile([1, N], F32, tag="st_x", bufs=1, name="sd")
        nc.scalar.activation(sd[0:1, :], var[0:1, :], AF.Sqrt,
                             bias=eps_t[0:1, 0:1])
        rstd = scratch.tile([1, N], F32R, tag="st_r", bufs=1, name="rstd")
        nc.vector.reciprocal(rstd[0:1, :], sd[0:1, :])
        ps_mu = tp_ps.tile([128, N], F32, tag="tp", name="ps_mu")
        nc.tensor.matmul(ps_mu[:, 0:N], ones_r[0:1, :], mu[0:1, :])
        ps_rs = tp_ps.tile([128, N], F32, tag="tp", name="ps_rs")
        nc.tensor.matmul(ps_rs[:, 0:N], ones_r[0:1, :], rstd[0:1, :])
        mu_sb = scratch.tile([128, N], F32R, tag="musb", bufs=2, name="mu_sb")
        nc.vector.tensor_copy(mu_sb[:, 0:N], ps_mu[:, 0:N])
        rs_sb = scratch.tile([128, N], F32R, tag="rssb", bufs=2, name="rs_sb")
        nc.vector.tensor_copy(rs_sb[:, 0:N], ps_rs[:, 0:N])
        eng = nc.gpsimd if xm_pool else nc.vector
        for ic in range(n_chunks):
            xm = scratch.tile([128, N], F32, tag="sq", name="xm")
            eng.tensor_sub(xm[:, 0:N], x_fn(ic), mu_sb[:, 0:N])
            eng.tensor_mul(xm[:, 0:N], xm[:, 0:N], rs_sb[:, 0:N])
            nc.scalar.activation(out_fn(ic), xm[:, 0:N], AF.Identity,
                                 bias=b_col[:, ic:ic + 1],
                                 scale=g_col[:, ic:ic + 1])

    def normalize_head(h, src, oT):
        """oT head slice = src[0:HD] / src[HD] (softmax sums row)."""
        off = (h % HPC) * HD
        fc = h // HPC
        rec = scratch.tile([1, NQ], F32R, tag="st_e", bufs=1, name="rec")
        nc.vector.reciprocal(rec[0:1, :], src[HD:HD + 1, :])
        ps_b = mm_ps.tile([HD, NQ], F32, tag="mm", name="ps_bc")
        nc.tensor.matmul(ps_b[0:HD, :], ones_r[0:1, 0:HD], rec[0:1, :])
        nc.vector.tensor_mul(oT[off:off + HD, fc, 0:NQ], src[0:HD, :],
                             ps_b[0:HD, :])

    def head_scores(b, h, kT, q2T, e_scale):
        """exp(scaled scores) for head h -> fp8 [128, 2*NQ] tile."""
        off = (h % HPC) * HD
        fc = h // HPC
        ps_s = sc_ps.tile([128, 2 * NQ], F32, tag="sc", name="ps_sc")
        for js in range(2):
            nc.tensor.matmul(
                ps_s[:, js * NQ:(js + 1) * NQ],
                kT[off:off + HD, fc, js * 128:(js + 1) * 128],
                q2T[off:off + HD, fc, b * NQ:(b + 1) * NQ])
        e = scratch.tile([128, 2, NQ], F8, tag="exp", bufs=6, name="e")
        nc.scalar.activation(
            e[:, :, :], ps_s[:, :].rearrange("p (two n) -> p two n", two=2),
            AF.Exp, scale=e_scale)
        return e

    # ================= P0 + self-attention =================
    x1T = persist.tile([128, WC, T], F32R, tag="xper", name="x1T")
    with tc.tile_pool(name="sa_w", bufs=1) as sa_w, \
         tc.tile_pool(name="sa", bufs=1) as sa:
        qT = sa.tile([128, WC, T], BF16, tag="qT", name="qT")
        qkT = sa.tile([128, WC, T], BF16, tag="big", bufs=3, name="qkT")
        for b in range(B_loc):
            q_tm = sa.tile([128, NQC, W], F32R, tag="tm", bufs=2, name="q_tm")
            nc.sync.dma_start(
                q_tm[:], q_flat[b * NQ:(b + 1) * NQ, :].rearrange(
                    "(c p) w -> p c w", p=128).bitcast(F32R))
            p_tm = sa.tile([128, NQC, W], F32R, tag="tm", bufs=2, name="p_tm")
            nc.sync.dma_start(
                p_tm[:], pe_flat[b * NQ:(b + 1) * NQ, :].rearrange(
                    "(c p) w -> p c w", p=128).bitcast(F32R))
            for fc in range(WC):
                transpose_group(
                    qT[:, fc, b * NQ:(b + 1) * NQ],
                    [q_tm[:, tcx, fc * 128:(fc + 1) * 128]
                     for tcx in range(NQC)])
            for tcx in range(NQC):
                nc.vector.tensor_add(q_tm[:, tcx, :], q_tm[:, tcx, :],
                                     p_tm[:, tcx, :])
            for fc in range(WC):
                transpose_group(
                    qkT[:, fc, b * NQ:(b + 1) * NQ],
                    [q_tm[:, tcx, fc * 128:(fc + 1) * 128]
                     for tcx in range(NQC)])

        wqt = load_wT(sa_w, "wt", dram['sa_wq'], W, W, "sa_wq_t",
                      wr_pool=sa_w, ev='act', bufs=2, dtype=BF16)
        qsaT = sa.tile([128, WC, T], BF16, tag="big", bufs=3, name="qsaT")
        for oc in range(WC):
            ps = mm_ps.tile([128, T], F32, tag="mm", name="ps_q")
            gemm(ps[:, 0:T], wqt, oc, lambda ic: qkT[:, ic, :], WC)
            nc.vector.tensor_copy(qsaT[:, oc, :], ps[:, 0:T])
        wkt = load_wT(sa_w, "wt", dram['sa_wk'], W, W, "sa_wk_t",
                      wr_pool=sa_w, ev='act', bufs=2, dtype=BF16)
        ksaT = sa.tile([128, WC, T], BF16, tag="big", bufs=3, name="ksaT")
        for oc in range(WC):
            ps = mm_ps.tile([128, T], F32, tag="mm", name="ps_k")
            gemm(ps[:, 0:T], wkt, oc, lambda ic: qkT[:, ic, :], WC)
            nc.vector.tensor_copy(ksaT[:, oc, :], ps[:, 0:T])
        wvt = load_wT(sa_w, "wt", dram['sa_wv'], W, W, "sa_wv_t",
                      wr_pool=sa_w, ev='act', bufs=2, dtype=BF16)
        vext_all = sa.tile([128, TC, NH, HD + 1], F8, tag="vext",
                           name="vext_sa")
        for tcx in range(TC):
            for oh in range(W // 512):
                ps = mm_ps.tile([128, 512], F32, tag="mm", name="ps_v")
                for ic in range(WC):
                    nc.tensor.matmul(
                        ps[:, 0:512],
                        qT[:, ic, tcx * 128:(tcx + 1) * 128],
                        wvt[:, ic, oh * 512:(oh + 1) * 512],
                        start=(ic == 0), stop=(ic == WC - 1))
                nh0 = oh * (512 // HD)
                nc.vector.tensor_copy(
                    vext_all[:, tcx, nh0:nh0 + 512 // HD, 0:HD],
                    ps[:, 0:512].rearrange("p (h d) -> p h d", d=HD))
            nc.gpsimd.memset(vext_all[:, tcx, :, HD], 1.0)
        wot = load_wT(sa_w, "wt", dram['sa_wo'], W, W, "sa_wo_t",
                      wr_pool=sa_w, ev='act', bufs=2, dtype=BF16)

        osaT = sa.tile([128, WC, NQ], BF16, tag="osaT", name="osaT")
        x1pre = sa.tile([128, WC, NQ], F32R, tag="x1pre", name="x1pre")
        for b in range(B_loc):
            vx = vext_all[:, b * NQC:(b + 1) * NQC, :, :]
            kTb = ksaT[:, :, b * NQ:(b + 1) * NQ]
            for hp in range(NH // 2):
                e0 = head_scores(b, 2 * hp, kTb, qsaT, SCALE)
                e1 = head_scores(b, 2 * hp + 1, kTb, qsaT, SCALE)
                ps_o = pv_ps.tile([HD + 1, 2, NQ], F32, tag="pv",
                                  name="ps_pv2")
                for sub, e in ((0, e0), (1, e1)):
                    nc.tensor.matmul(ps_o[0:HD + 1, sub, :],
                                     vx[:, :, 2 * hp + sub, :], e[:, :, :],
                                     perf_mode=DR)
                pv_sb = scratch.tile([HD + 1, 2, NQ], F32, tag="pvsb",
                                     bufs=2, name="pv_sb")
                nc.vector.tensor_copy(pv_sb[0:HD + 1, :, :],
                                      ps_o[0:HD + 1, :, :])
                for sub in range(2):
                    normalize_head(2 * hp + sub, pv_sb[:, sub, :], osaT)
            for oc in range(WC):
                ps = mm_ps.tile([128, NQ], F32, tag="mm", name="ps_o")
                gemm(ps[:, 0:NQ], wot, oc, lambda ic: osaT[:, ic, :], WC)
                nc.vector.tensor_add(x1pre[:, oc, :], ps[:, 0:NQ],
                                     qT[:, oc, b * NQ:(b + 1) * NQ])
            layernorm(lambda ic: x1pre[:, ic, :], WC, NQ,
                      cols['ln1_g'], cols['ln1_b'],
                      lambda ic: x1T[:, ic, b * NQ:(b + 1) * NQ])

    # ================= cross-attention =================
    with tc.tile_pool(name="ca_w", bufs=1) as ca_w, \
         tc.tile_pool(name="ca", bufs=1) as ca:
        q2T = ca.tile([128, WC, T], BF16, tag="q2T", name="q2T")
        wvt2 = load_wT(ca_w, "wt8B", dram['ca_wv'], W, W, "ca_wv_t",
                       dtype=F8, scale16=True, ev='act')
        with tc.tile_pool(name="ca_early", bufs=1) as cae:
            x1pT = cae.tile([128, WC, T], BF16, tag="x1pT", name="x1pT")
            for b in range(B_loc):
                p_tm = cae.tile([128, NQC, W], F32R, tag="ptm", bufs=2,
                                name="p_tm2")
                nc.sync.dma_start(
                    p_tm[:], pe_flat[b * NQ:(b + 1) * NQ, :].rearrange(
                        "(c p) w -> p c w", p=128).bitcast(F32R))
                peT = cae.tile([128, WC, NQ], F32R, tag="peT", name="peT")
                for fc in range(WC):
                    transpose_group(
                        peT[:, fc, 0:NQ],
                        [p_tm[:, tcx, fc * 128:(fc + 1) * 128]
                         for tcx in range(NQC)])
                for fc in range(WC):
                    nc.vector.tensor_add(
                        x1pT[:, fc, b * NQ:(b + 1) * NQ],
                        x1T[:, fc, b * NQ:(b + 1) * NQ], peT[:, fc, :])
            wqt2 = load_wT(ca_w, "wtA", dram['ca_wq'], W, W, "ca_wq_t",
                           ev='act', dtype=BF16)
            for oc in range(WC):
                ps = mm_ps.tile([128, T], F32, tag="mm", name="ps_q2")
                gemm(ps[:, 0:T], wqt2, oc, lambda ic: x1pT[:, ic, :], WC)
                nc.vector.tensor_copy(q2T[:, oc, :], ps[:, 0:T])

        wkt2 = load_wT(ca_w, "wt8A", dram['ca_wk'], W, W, "ca_wk_t",
                       dtype=F8, scale16=True, ev='act')
        wot2 = load_wT(ca_w, "wtA", dram['ca_wo'], W, W, "ca_wo_t",
                       ev='act', dtype=BF16)

        ocaT = ca.tile([128, WC, T], BF16, tag="ocaT", name="ocaT")
        x2T = persist.tile([128, WC, T], F32R, tag="xper2", name="x2T")
        with tc.tile_pool(name="ca_acc", bufs=1) as cacc, \
             tc.tile_pool(name="ca_jc", bufs=1) as cjc:
            for b in range(B_loc):
                acc = cacc.tile([HD + 1, NH, NQ], F32, tag="acc",
                                name="acc_ca")
                for g in range(NJC // G):
                    k2Ts, vexts = [], []
                    for jg in range(G):
                        jc = g * G + jg
                        tok0 = b * S + jc * JC
                        m_tm = []
                        for sj in range(JSC):
                            mt = cjc.tile([128, W], F32R, tag="m_tm", bufs=3,
                                          name="m_tm")
                            nc.sync.dma_start(
                                mt[:],
                                m_flat[tok0 + sj * 128:
                                       tok0 + (sj + 1) * 128, :]
                                .bitcast(F32R))
                            m_tm.append(mt)
                        mT = cjc.tile([128, WC, JC], F8, tag="mT",
                                      bufs=2, name="mT")
                        for fp_ in range(WC // 2):
                            transpose_group(
                                mT[:, fp_ * 2:fp_ * 2 + 2, 0:JC],
                                [m_tm[sj][:, fc * 128:(fc + 1) * 128]
                                 for fc in (fp_ * 2, fp_ * 2 + 1)
                                 for sj in range(JSC)], n2=JC)
                        k2T = cjc.tile([128, WC, JC], BF16, tag="k2T",
                                       bufs=2 * G - 1, name="k2T")
                        for op_ in range(WC // 2):
                            ps = mm_ps.tile([128, 2, JC], F32, tag="mm",
                                            name="ps_k2")
                            for sub in range(2):
                                oc = op_ * 2 + sub
                                for icp in range(WC // 2):
                                    nc.tensor.matmul(
                                        ps[:, sub, :],
                                        wkt2[:, 2 * icp:2 * icp + 2,
                                             oc * 128:(oc + 1) * 128],
                                        mT[:, 2 * icp:2 * icp + 2, :],
                                        start=(icp == 0),
                                        stop=(icp == WC // 2 - 1),
                                        perf_mode=DR)
                            nc.vector.tensor_copy(
                                k2T[:, op_ * 2:op_ * 2 + 2, :],
                                ps[:, :, 0:JC])
                        vext = cjc.tile([128, JSC, NH, HD + 1], F8,
                                        tag="vext", bufs=2 * G, name="vext_ca")
                        for sj in range(JSC):
                            for op2 in range(W // 512):
                                ps = mm_ps.tile([128, 2, 256], F32, tag="mm",
                                                name="ps_v2")
                                for half in range(2):
                                    oh = op2 * 2 + half
                                    for icp in range(WC // 2):
                                        nc.tensor.matmul(
                                            ps[:, half, :],
                                            mT[:, 2 * icp:2 * icp + 2,
                                               sj * 128:(sj + 1) * 128],
                                            wvt2[:, 2 * icp:2 * icp + 2,
                                                 oh * 256:(oh + 1) * 256],
                                            start=(icp == 0),
                                            stop=(icp == WC // 2 - 1),
                                            perf_mode=DR)
                                nh0 = op2 * (512 // HD)
                                nc.vector.tensor_copy(
                                    vext[:, sj, nh0:nh0 + 512 // HD, 0:HD],
                                    ps[:, :, :].rearrange(
                                        "p a (h d) -> p (a h) d", d=HD))
                            nc.gpsimd.memset(vext[:, sj, :, HD], W8SCALE)
                        k2Ts.append(k2T)
                        vexts.append(vext)
                    for hp in range(NH // 2):
                        ps_o = pv_ps.tile([HD + 1, 2, NQ], F32, tag="pv",
                                          name="ps_pv2")
                        for jg in range(G):
                            es = [head_scores(b, 2 * hp + sub, k2Ts[jg],
                                              q2T, SCALE / W8SCALE)
                                  for sub in range(2)]
                            for sub in range(2):
                                nc.tensor.matmul(
                                    ps_o[0:HD + 1, sub, :],
                                    vexts[jg][:, :, 2 * hp + sub, :],
                                    es[sub][:, :, :],
                                    start=(jg == 0), stop=(jg == G - 1),
                                    perf_mode=DR)
                        if g == 0:
                            nc.vector.tensor_copy(
                                acc[0:HD + 1, 2 * hp:2 * hp + 2, :],
                                ps_o[0:HD + 1, :, :])
                        else:
                            nc.vector.tensor_add(
                                acc[0:HD + 1, 2 * hp:2 * hp + 2, :],
                                acc[0:HD + 1, 2 * hp:2 * hp + 2, :],
                                ps_o[0:HD + 1, :, :])
                for h in range(NH):
                    normalize_head(h, acc[:, h, :],
                                   ocaT[:, :, b * NQ:(b + 1) * NQ])
                x2pre = cacc.tile([128, WC, NQ], F32R, tag="x2pre", bufs=2,
                                  name="x2pre")
                for oc in range(WC):
                    ps = mm_ps.tile([128, NQ], F32, tag="mm", name="ps_o2")
                    gemm(ps[:, 0:NQ], wot2, oc,
                         lambda ic: ocaT[:, ic, b * NQ:(b + 1) * NQ], WC)
                    nc.vector.tensor_add(x2pre[:, oc, :], ps[:, 0:NQ],
                                         x1T[:, oc, b * NQ:(b + 1) * NQ])
                layernorm(lambda ic: x2pre[:, ic, :], WC, NQ,
                          cols['ln2_g'], cols['ln2_b'],
                          lambda ic: x2T[:, ic, b * NQ:(b + 1) * NQ])

    # ================= FFN =================
    with tc.tile_pool(name="ffn", bufs=1) as ffn:
        hT = ffn.tile([128, MC, T], BF16, tag="hT", name="hT")
        for oc in range(MC):
            w1t = ffn.tile([128, WC, 128], F32R, tag="w1t", bufs=2, name="w1t")
            wr = ffn.tile([128, W], F32R, tag="w1row", bufs=3, name="wr1")
            nc.sync.dma_start(
                wr[:], dram['ffn_w1'][oc * 128:(oc + 1) * 128, :].bitcast(F32R))
            for half in range(W // 512):
                transpose_group(
                    w1t[:, half * 4:(half + 1) * 4, :],
                    [wr[:, half * 512 + k * 128:half * 512 + (k + 1) * 128]
                     for k in range(4)])
            ps = mm_ps.tile([128, T], F32, tag="mm", name="ps_h")
            for ic in range(WC):
                nc.tensor.matmul(ps[:, 0:T], w1t[:, ic, :], x2T[:, ic, :],
                                 start=(ic == 0), stop=(ic == WC - 1))
            nc.scalar.activation(hT[:, oc, :], ps[:, 0:T], AF.Relu,
                                 bias=b1_col[:, oc:oc + 1])
        x3pre = ffn.tile([128, WC, T], F32R, tag="x3pre", name="x3pre")
        for oc in range(WC):
            w2t = ffn.tile([128, MC, 128], BF16, tag="w2t", bufs=2, name="w2t")
            PIECE = 1024 if MLP % 1024 == 0 else 512
            for piece in range(MLP // PIECE):
                wr = ffn.tile([128, PIECE], F32R, tag="w2row", bufs=3,
                              name="wr2", padded_shape=[128, 1024])
                nc.sync.dma_start(
                    wr[:], dram['ffn_w2'][oc * 128:(oc + 1) * 128,
                                          piece * PIECE:(piece + 1) * PIECE]
                    .bitcast(F32R))
                for hh in range(PIECE // 512):
                    half = piece * (PIECE // 512) + hh
                    transpose_group(
                        w2t[:, half * 4:(half + 1) * 4, :],
                        [wr[:, hh * 512 + k * 128:hh * 512 + (k + 1) * 128]
                         for k in range(4)])
            ps = mm_ps.tile([128, T], F32, tag="mm", name="ps_f")
            for ic in range(MC):
                nc.tensor.matmul(ps[:, 0:T], w2t[:, ic, :], hT[:, ic, :],
                                 start=(ic == 0), stop=(ic == MC - 1))
            tmp = scratch.tile([128, T], F32, tag="sq", name="f_tmp")
            nc.scalar.activation(tmp[:, 0:T], ps[:, 0:T], AF.Identity,
                                 bias=cols['ffn_b2'][:, oc:oc + 1])
            nc.vector.tensor_add(x3pre[:, oc, :], tmp[:, 0:T], x2T[:, oc, :])
        x3T = ffn.tile([128, WC, T], F32, tag="x3T", name="x3T")
        for b in range(B_loc):
            n0 = b * NQ
            layernorm(lambda ic: x3pre[:, ic, n0:n0 + NQ], WC, NQ,
                      cols['ln3_g'], cols['ln3_b'],
                      lambda ic: x3T[:, ic, n0:n0 + NQ], xm_pool=False)
            for tcx in range(b * NQC, (b + 1) * NQC):
                o_tm = ffn.tile([128, W], F32, tag="o_tm", bufs=2, name="o_tm")
                for g in range(WC // 4):
                    transpose_group(
                        o_tm[:, g * 512:(g + 1) * 512],
                        [x3T[:, g * 4 + k, tcx * 128:(tcx + 1) * 128]
                         for k in range(4)], exact=True)
                nc.sync.dma_start(out_flat[tcx * 128:(tcx + 1) * 128, :],
                                  o_tm[:])

    return out_d


_PROGRAM_CACHE = {}


def _get_program(B_loc, NQ, S, W, NH, MLP, JC=256, repeat=1):
    key = (B_loc, NQ, S, W, NH, MLP, JC, repeat)
    if key not in _PROGRAM_CACHE:
        nc = bacc.Bacc("TRN2", target_bir_lowering=False, debug=False)
        with tile.TileContext(nc) as tc, \
             nc.allow_low_precision(reason="fp32r/fp8 matmul pipeline"):
            for r in range(repeat):
                with ExitStack() as ctx:
                    build_decoder(nc, tc, ctx, B_loc, NQ, S, W, NH, MLP, JC,
                                  suffix=("" if r == 0 else f"_r{r}"))
        nc.compile()
        _PROGRAM_CACHE[key] = nc
    return _PROGRAM_CACHE[key]


def kernel(**inputs):
    B, NQ, W = inputs['query'].shape
    S = inputs['enc_mem'].shape[1]
    MLP = inputs['ffn_w1'].shape[0]
    NH = 16
    assert B % N_CORES == 0
    B_loc = B // N_CORES

    nc = _get_program(B_loc, NQ, S, W, NH, MLP)

    shard_names = {'query', 'enc_mem', 'out_pos_enc'}
    in_maps = []
    for c in range(N_CORES):
        m = {}
        for k, v in inputs.items():
            v = np.ascontiguousarray(np.asarray(v, dtype=np.float32))
            if k in shard_names:
                m[k] = np.ascontiguousarray(v[c * B_loc:(c + 1) * B_loc])
            else:
                m[k] = v
        in_maps.append(m)

    res = run_bass_kernel_spmd(nc, in_maps, list(range(N_CORES)))
    return np.concatenate([res.results[c]["out"] for c in range(N_CORES)],
                          axis=0)


# revision 33
# speedup vs baseline: 1.2994x; 1.2994x over previous
"""Trainium2 Bass kernel for nn_DecoderLayer (dense transformer decoder layer).

Sharding: data-parallel over batch (16 batches -> 8 cores x 2 each). Each core
runs the full decoder layer on its batch slice; no collectives.

Layout: activations are kept feature-major ("xT": [feature partitions, token
free]) so every linear is psum = W^T.T @ xT on the PE with fp32r inputs
(1 cycle/row at N>=256). Weights stream from DRAM in natural [O, I] layout and
are transposed on-chip with PE-transpose blocks. Attention uses transposed
scores S^T = K_hT.T @ Q_hT ([j partitions, i free]), exp without
max-subtraction (scores are bounded, |s|*scale < ~2), and a ones-column
appended to V so the softmax denominator comes out of the same PV matmul.
LayerNorm runs feature-major with partition sums via ones-vector matmuls and
per-token broadcast via K=1 matmuls.

fp8 fast path: the cross-attention K/V projections (the dominant GEMMs, over
S=4096 enc tokens) and all PV matmuls run in fp8e4 with the PE's DoubleRow
mode (two K-blocks contracted per instruction at 0.5 cycles/row = 4x fp32r).
Quantization noise in K, V, and the exp'd scores sits entirely behind the
softmax average over 4096 (resp. 256) positions, so it attenuates by ~64x
(resp. 16x) before reaching the residual stream. The K-projection weights are
pre-scaled by 16 (folded into the PE transpose via a scaled identity) to
dodge fp8 subnormals, and the exp scale divides it back out. The V path's
16x rides through PV and is cancelled by a 16.0 ones-column (the softmax
denominator picks up the same factor). Scores themselves stay fp32r.

Cross-attention runs in jc-groups of 4 chunks whose PV partials accumulate
in PSUM across the group, quartering the DVE accumulate traffic.
"""
import sys
import numpy as np

sys.path.insert(0, '/opt/trn_rl_repo')

import concourse.bass as bass  # noqa: E402
import concourse.tile as tile  # noqa: E402
from concourse import bacc, mybir  # noqa: E402
from concourse.bass_utils import run_bass_kernel_spmd  # noqa: E402
from concourse.masks import make_identity  # noqa: E402
from contextlib import ExitStack  # noqa: E402

F32 = mybir.dt.float32
F32R = mybir.dt.float32r
F8 = mybir.dt.float8e4
BF16 = mybir.dt.bfloat16
AF = mybir.ActivationFunctionType
DR = mybir.MatmulPerfMode.DoubleRow

EPS = 1e-5
N_CORES = 8
W8SCALE = 16.0


def build_decoder(nc, tc, ctx, B_loc, NQ, S, W, NH, MLP, JC=256, suffix=""):
    HD = W // NH
    assert HD == 64 and NQ % 128 == 0 and W % 512 == 0 and JC % 128 == 0
    T = B_loc * NQ          # decoder tokens per core
    TC = T // 128
    WC = W // 128
    MC = MLP // 128
    NJC = S // JC           # enc chunks per batch
    JSC = JC // 128
    NQC = NQ // 128
    SCALE = float(W) ** -0.5
    HPC = 128 // HD         # heads per feature chunk (2)
    G = 4                   # jc chunks per PV-accumulation group
    assert NJC % G == 0 and JSC == 2 and NQC == 2

    dram = {}
    for name, shape in [
        ('query', [B_loc, NQ, W]), ('enc_mem', [B_loc, S, W]),
        ('out_pos_enc', [B_loc, NQ, W]),
        ('sa_wq', [W, W]), ('sa_wk', [W, W]), ('sa_wv', [W, W]), ('sa_wo', [W, W]),
        ('ca_wq', [W, W]), ('ca_wk', [W, W]), ('ca_wv', [W, W]), ('ca_wo', [W, W]),
        ('ffn_w1', [MLP, W]), ('ffn_b1', [MLP]), ('ffn_w2', [W, MLP]), ('ffn_b2', [W]),
        ('ln1_g', [W]), ('ln1_b', [W]), ('ln2_g', [W]), ('ln2_b', [W]),
        ('ln3_g', [W]), ('ln3_b', [W]),
    ]:
        if suffix:
            dram[name] = build_decoder._dram_cache[name]
        else:
            dram[name] = nc.dram_tensor(name, shape, F32, kind="ExternalInput")
    build_decoder._dram_cache = dict(dram)
    out_d = nc.dram_tensor("out" + suffix, [B_loc, NQ, W], F32,
                           kind="ExternalOutput")

    q_flat = dram['query'].rearrange("b n w -> (b n) w")
    pe_flat = dram['out_pos_enc'].rearrange("b n w -> (b n) w")
    m_flat = dram['enc_mem'].rearrange("b s w -> (b s) w")
    out_flat = out_d.rearrange("b n w -> (b n) w")

    # ---------------- global pools ----------------
    consts = ctx.enter_context(tc.tile_pool(name="consts", bufs=1))
    ca_w = ctx.enter_context(tc.tile_pool(name="ca_w", bufs=1))
    persist = ctx.enter_context(tc.tile_pool(name="persist", bufs=1))
    scratch = ctx.enter_context(tc.tile_pool(name="scratch", bufs=2))
    tp_ps = ctx.enter_context(tc.tile_pool(name="tp_ps", bufs=2, space="PSUM"))
    mm_ps = ctx.enter_context(tc.tile_pool(name="mm_ps", bufs=2, space="PSUM"))
    sc_ps = ctx.enter_context(tc.tile_pool(name="sc_ps", bufs=3, space="PSUM"))
    pv_ps = ctx.enter_context(tc.tile_pool(name="pv_ps", bufs=1, space="PSUM"))

    ident = consts.tile([128, 128], F32, tag="ident")
    make_identity(nc, ident[:])
    ones_f = consts.tile([128, 128], F32, tag="ones_f")
    nc.gpsimd.memset(ones_f[:], 1.0)
    ones_r = consts.tile([128, 128], F32R, tag="ones_r")
    nc.vector.tensor_copy(ones_r[:], ones_f[:])
    ident_r = consts.tile([128, 128], F32R, tag="ident_r")
    nc.vector.tensor_copy(ident_r[:], ident[:])
    ident16 = consts.tile([128, 128], F32R, tag="ident16")
    nc.vector.tensor_scalar_mul(ident16[:], ident[:], W8SCALE)
    eps_t = consts.tile([1, 1], F32, tag="eps")
    nc.gpsimd.memset(eps_t[:], EPS)

    def load_col(name, n):
        """[n] param vector -> [128, n/128] per-partition columns."""
        nch = n // 128
        land = scratch.tile([128, 128], F32, tag="colland", bufs=2,
                            name=name + "_land")
        nc.sync.dma_start(land[0:nch, :],
                          dram[name].rearrange("(c p) -> c p", p=128))
        pt = tp_ps.tile([128, 512], F32, tag="tp", name="pt_col")
        nc.tensor.transpose(pt[:, 0:128], land[:, 0:128], ident[:])
        t = consts.tile([128, nch], F32, tag=name, name=name + "_col")
        nc.vector.tensor_copy(t[:], pt[:, 0:nch])
        return t
    cols = {k: load_col(k, W) for k in
            ['ln1_g', 'ln1_b', 'ln2_g', 'ln2_b', 'ln3_g', 'ln3_b', 'ffn_b2']}
    b1_col = load_col('ffn_b1', MLP)

    # ---------------- helpers ----------------
    def evict(engine, dst, src):
        if engine == 'act':
            nc.scalar.activation(dst, src, AF.Copy)
        else:
            nc.vector.tensor_copy(dst, src)

    def transpose_group(dst_slice, src_slices, exact=False, scale16=False,
                        ev='dve', n2=128):
        """Transpose up to 4 [128,128] blocks through one PSUM bank, evict
        with a single copy on `ev` engine. fp32r by default; scale16
        multiplies the transpose by W8SCALE (for fp8 weight tiles)."""
        n = len(src_slices)
        dt = F32 if exact else F32R
        idn = ident if exact else (ident16 if scale16 else ident_r)
        pt = tp_ps.tile([128, 512], dt, tag="tp", name="ptg")
        for i, src in enumerate(src_slices):
            s = src if exact else src.bitcast(F32R)
            nc.tensor.transpose(pt[:, i * 128:(i + 1) * 128], s, idn[:])
        src_view = pt[:, 0:n * 128]
        if len(dst_slice.shape) == 3:
            src_view = src_view.rearrange("p (c n2) -> p c n2", n2=n2)
        evict(ev, dst_slice, src_view)

    def load_wT(pool, tag, w_dram, O, I, name, wr_pool=None, dtype=F32R,
                scale16=False, ev='dve', bufs=1):
        """Stream W [O, I] from DRAM -> W^T tile [128, I/128, O] (dtype)."""
        wt = pool.tile([128, I // 128, O], dtype, tag=tag, name=name,
                       bufs=bufs)
        for ob in range(O // 128):
            if wr_pool is not None:
                wr = wr_pool.tile([128, I], F32R, tag="wrow_big", bufs=4,
                                  name="wrb")
                nc.sync.dma_start(
                    wr[:], w_dram[ob * 128:(ob + 1) * 128, :].bitcast(F32R))
                for half in range(I // 512):
                    transpose_group(
                        wt[:, half * 4:(half + 1) * 4,
                           ob * 128:(ob + 1) * 128],
                        [wr[:, half * 512 + k * 128:half * 512 + (k + 1) * 128]
                         for k in range(4)], scale16=scale16, ev=ev)
            else:
                for half in range(I // 512):
                    wr = scratch.tile([128, 512], F32R, tag="wrow", bufs=5,
                                      name="wr")
                    nc.sync.dma_start(
                        wr[:], w_dram[ob * 128:(ob + 1) * 128,
                                      half * 512:(half + 1) * 512]
                        .bitcast(F32R))
                    transpose_group(
                        wt[:, half * 4:(half + 1) * 4,
                           ob * 128:(ob + 1) * 128],
                        [wr[:, k * 128:(k + 1) * 128] for k in range(4)],
                        scale16=scale16, ev=ev)
        return wt

    def gemm(psum, wt, oc, rhs_fn, ICn):
        for ic in range(ICn):
            nc.tensor.matmul(psum, wt[:, ic, oc * 128:(oc + 1) * 128],
                             rhs_fn(ic), start=(ic == 0), stop=(ic == ICn - 1))

    def layernorm(x_fn, n_chunks, N, g_col, b_col, out_fn, xm_pool=True):
        """Feature-major LN over the partition (feature) dim."""
        ps_s = tp_ps.tile([1, N], F32, tag="tp", name="ps_s")
        for ic in range(n_chunks):
            nc.tensor.matmul(ps_s[0:1, :], ones_r[:, 0:1], x_fn(ic),
                             start=(ic == 0), stop=(ic == n_chunks - 1))
        ps_q = tp_ps.tile([1, N], F32, tag="tp", name="ps_q")
        for ic in range(n_chunks):
            sq = scratch.tile([128, N], F32R, tag="sq", name="sq")
            nc.gpsimd.tensor_mul(sq[:, 0:N], x_fn(ic), x_fn(ic))
            nc.tensor.matmul(ps_q[0:1, :], ones_r[:, 0:1], sq[:, 0:N],
                             start=(ic == 0), stop=(ic == n_chunks - 1))
        inv_w = 1.0 / (n_chunks * 128)
        mu = scratch.tile([1, N], F32R, tag="st_mu", bufs=1, name="mu")
        nc.scalar.activation(mu[0:1, :], ps_s[0:1, :], AF.Copy, scale=inv_w)
        ex2 = scratch.tile([1, N], F32, tag="st_e", bufs=1, name="ex2")
        nc.scalar.activation(ex2[0:1, :], ps_q[0:1, :], AF.Copy, scale=inv_w)
        mu2 = scratch.tile([1, N], F32, tag="st_x", bufs=1, name="mu2")
        nc.vector.tensor_mul(mu2[0:1, :], mu[0:1, :], mu[0:1, :])
        var = scratch.tile([1, N], F32, tag="st_v", bufs=1, name="var")
        nc.vector.tensor_sub(var[0:1, :], ex2[0:1, :], mu2[0:1, :])
        sd = scratch.tile([1, N], F32, tag="st_x", bufs=1, name="sd")
        nc.scalar.activation(sd[0:1, :], var[0:1, :], AF.Sqrt,
                             bias=eps_t[0:1, 0:1])
        rstd = scratch.tile([1, N], F32R, tag="st_r", bufs=1, name="rstd")
        nc.vector.reciprocal(rstd[0:1, :], sd[0:1, :])
        ps_mu = tp_ps.tile([128, N], F32, tag="tp", name="ps_mu")
        nc.tensor.matmul(ps_mu[:, 0:N], ones_r[0:1, :], mu[0:1, :])
        ps_rs = tp_ps.tile([128, N], F32, tag="tp", name="ps_rs")
        nc.tensor.matmul(ps_rs[:, 0:N], ones_r[0:1, :], rstd[0:1, :])
        mu_sb = scratch.tile([128, N], F32R, tag="musb", bufs=2, name="mu_sb")
        nc.vector.tensor_copy(mu_sb[:, 0:N], ps_mu[:, 0:N])
        rs_sb = scratch.tile([128, N], F32R, tag="rssb", bufs=2, name="rs_sb")
        nc.vector.tensor_copy(rs_sb[:, 0:N], ps_rs[:, 0:N])
        eng = nc.gpsimd if xm_pool else nc.vector
        for ic in range(n_chunks):
            xm = scratch.tile([128, N], F32, tag="sq", name="xm")
            eng.tensor_sub(xm[:, 0:N], x_fn(ic), mu_sb[:, 0:N])
            eng.tensor_mul(xm[:, 0:N], xm[:, 0:N], rs_sb[:, 0:N])
            nc.scalar.activation(out_fn(ic), xm[:, 0:N], AF.Identity,
                                 bias=b_col[:, ic:ic + 1],
                                 scale=g_col[:, ic:ic + 1])

    def normalize_head(h, src, oT):
        """oT head slice = src[0:HD] / src[HD] (softmax sums row)."""
        off = (h % HPC) * HD
        fc = h // HPC
        rec = scratch.tile([1, NQ], F32R, tag="st_e", bufs=1, name="rec")
        nc.vector.reciprocal(rec[0:1, :], src[HD:HD + 1, :])
        ps_b = mm_ps.tile([HD, NQ], F32, tag="mm", name="ps_bc")
        nc.tensor.matmul(ps_b[0:HD, :], ones_r[0:1, 0:HD], rec[0:1, :])
        nc.vector.tensor_mul(oT[off:off + HD, fc, 0:NQ], src[0:HD, :],
                             ps_b[0:HD, :])

    def head_scores(b, h, kT, q2T, e_scale):
        """exp(scaled scores) for head h -> fp8 [128, 2*NQ] tile."""
        off = (h % HPC) * HD
        fc = h // HPC
        ps_s = sc_ps.tile([128, 2 * NQ], F32, tag="sc", name="ps_sc")
        for js in range(2):
            nc.tensor.matmul(
                ps_s[:, js * NQ:(js + 1) * NQ],
                kT[off:off + HD, fc, js * 128:(js + 1) * 128],
                q2T[off:off + HD, fc, b * NQ:(b + 1) * NQ])
        e = scratch.tile([128, 2, NQ], F8, tag="exp", bufs=6, name="e")
        nc.scalar.activation(
            e[:, :, :], ps_s[:, :].rearrange("p (two n) -> p two n", two=2),
            AF.Exp, scale=e_scale)
        return e

    # ================= P0 + self-attention =================
    x1T = persist.tile([128, WC, T], F32R, tag="xper", name="x1T")
    with tc.tile_pool(name="sa_w", bufs=1) as sa_w, \
         tc.tile_pool(name="sa", bufs=1) as sa:
        qT = sa.tile([128, WC, T], BF16, tag="qT", name="qT")
        qkT = sa.tile([128, WC, T], BF16, tag="big", bufs=3, name="qkT")
        for b in range(B_loc):
            q_tm = sa.tile([128, NQC, W], F32R, tag="tm", bufs=2, name="q_tm")
            nc.sync.dma_start(
                q_tm[:], q_flat[b * NQ:(b + 1) * NQ, :].rearrange(
                    "(c p) w -> p c w", p=128).bitcast(F32R))
            p_tm = sa.tile([128, NQC, W], F32R, tag="tm", bufs=2, name="p_tm")
            nc.sync.dma_start(
                p_tm[:], pe_flat[b * NQ:(b + 1) * NQ, :].rearrange(
                    "(c p) w -> p c w", p=128).bitcast(F32R))
            for fc in range(WC):
                transpose_group(
                    qT[:, fc, b * NQ:(b + 1) * NQ],
                    [q_tm[:, tcx, fc * 128:(fc + 1) * 128]
                     for tcx in range(NQC)])
            for tcx in range(NQC):
                nc.vector.tensor_add(q_tm[:, tcx, :], q_tm[:, tcx, :],
                                     p_tm[:, tcx, :])
            for fc in range(WC):
                transpose_group(
                    qkT[:, fc, b * NQ:(b + 1) * NQ],
                    [q_tm[:, tcx, fc * 128:(fc + 1) * 128]
                     for tcx in range(NQC)])

        wqt = load_wT(sa_w, "wt", dram['sa_wq'], W, W, "sa_wq_t",
                      wr_pool=sa_w, ev='act', bufs=2, dtype=BF16)
        qsaT = sa.tile([128, WC, T], BF16, tag="big", bufs=3, name="qsaT")
        for oc in range(WC):
            ps = mm_ps.tile([128, T], F32, tag="mm", name="ps_q")
            gemm(ps[:, 0:T], wqt, oc, lambda ic: qkT[:, ic, :], WC)
            nc.vector.tensor_copy(qsaT[:, oc, :], ps[:, 0:T])
        wkt = load_wT(sa_w, "wt", dram['sa_wk'], W, W, "sa_wk_t",
                      wr_pool=sa_w, ev='act', bufs=2, dtype=BF16)
        ksaT = sa.tile([128, WC, T], BF16, tag="big", bufs=3, name="ksaT")
        for oc in range(WC):
            ps = mm_ps.tile([128, T], F32, tag="mm", name="ps_k")
            gemm(ps[:, 0:T], wkt, oc, lambda ic: qkT[:, ic, :], WC)
            nc.vector.tensor_copy(ksaT[:, oc, :], ps[:, 0:T])
        wvt = load_wT(sa_w, "wt", dram['sa_wv'], W, W, "sa_wv_t",
                      wr_pool=sa_w, ev='act', bufs=2, dtype=BF16)
        vext_all = sa.tile([128, TC, NH, HD + 1], F8, tag="vext",
                           name="vext_sa")
        for tcx in range(TC):
            for oh in range(W // 512):
                ps = mm_ps.tile([128, 512], F32, tag="mm", name="ps_v")
                for ic in range(WC):
                    nc.tensor.matmul(
                        ps[:, 0:512],
                        qT[:, ic, tcx * 128:(tcx + 1) * 128],
                        wvt[:, ic, oh * 512:(oh + 1) * 512],
                        start=(ic == 0), stop=(ic == WC - 1))
                nh0 = oh * (512 // HD)
                nc.vector.tensor_copy(
                    vext_all[:, tcx, nh0:nh0 + 512 // HD, 0:HD],
                    ps[:, 0:512].rearrange("p (h d) -> p h d", d=HD))
            nc.gpsimd.memset(vext_all[:, tcx, :, HD], 1.0)
        wot = load_wT(sa_w, "wt", dram['sa_wo'], W, W, "sa_wo_t",
                      wr_pool=sa_w, ev='act', bufs=2, dtype=BF16)

        osaT = sa.tile([128, WC, NQ], BF16, tag="osaT", name="osaT")
        x1pre = sa.tile([128, WC, NQ], F32R, tag="x1pre", name="x1pre")
        for b in range(B_loc):
            vx = vext_all[:, b * NQC:(b + 1) * NQC, :, :]
            kTb = ksaT[:, :, b * NQ:(b + 1) * NQ]
            for hp in range(NH // 2):
                e0 = head_scores(b, 2 * hp, kTb, qsaT, SCALE)
                e1 = head_scores(b, 2 * hp + 1, kTb, qsaT, SCALE)
                ps_o = pv_ps.tile([HD + 1, 2, NQ], F32, tag="pv",
                                  name="ps_pv2")
                for sub, e in ((0, e0), (1, e1)):
                    nc.tensor.matmul(ps_o[0:HD + 1, sub, :],
                                     vx[:, :, 2 * hp + sub, :], e[:, :, :],
                                     perf_mode=DR)
                pv_sb = scratch.tile([HD + 1, 2, NQ], F32, tag="pvsb",
                                     bufs=2, name="pv_sb")
                nc.vector.tensor_copy(pv_sb[0:HD + 1, :, :],
                                      ps_o[0:HD + 1, :, :])
                for sub in range(2):
                    normalize_head(2 * hp + sub, pv_sb[:, sub, :], osaT)
            for oc in range(WC):
                ps = mm_ps.tile([128, NQ], F32, tag="mm", name="ps_o")
                gemm(ps[:, 0:NQ], wot, oc, lambda ic: osaT[:, ic, :], WC)
                nc.vector.tensor_add(x1pre[:, oc, :], ps[:, 0:NQ],
                                     qT[:, oc, b * NQ:(b + 1) * NQ])
            layernorm(lambda ic: x1pre[:, ic, :], WC, NQ,
                      cols['ln1_g'], cols['ln1_b'],
                      lambda ic: x1T[:, ic, b * NQ:(b + 1) * NQ],
                      xm_pool=(b == 0))

    # ================= cross-attention =================
    with tc.tile_pool(name="ca", bufs=1) as ca:
        q2T = ca.tile([128, WC, T], BF16, tag="q2T", name="q2T")
        wvt2 = load_wT(ca_w, "wt8B", dram['ca_wv'], W, W, "ca_wv_t",
                       dtype=F8, scale16=True, ev='act')
        with tc.tile_pool(name="ca_early", bufs=1) as cae:
            x1pT = cae.tile([128, WC, T], BF16, tag="x1pT", name="x1pT")
            for b in range(B_loc):
                p_tm = cae.tile([128, NQC, W], F32R, tag="ptm", bufs=2,
                                name="p_tm2")
                nc.sync.dma_start(
                    p_tm[:], pe_flat[b * NQ:(b + 1) * NQ, :].rearrange(
                        "(c p) w -> p c w", p=128).bitcast(F32R))
                peT = cae.tile([128, WC, NQ], F32R, tag="peT", name="peT")
                for fc in range(WC):
                    transpose_group(
                        peT[:, fc, 0:NQ],
                        [p_tm[:, tcx, fc * 128:(fc + 1) * 128]
                         for tcx in range(NQC)])
                for fc in range(WC):
                    nc.vector.tensor_add(
                        x1pT[:, fc, b * NQ:(b + 1) * NQ],
                        x1T[:, fc, b * NQ:(b + 1) * NQ], peT[:, fc, :])
            wqt2 = load_wT(ca_w, "wtA", dram['ca_wq'], W, W, "ca_wq_t",
                           ev='act', dtype=BF16)
            for oc in range(WC):
                ps = mm_ps.tile([128, T], F32, tag="mm", name="ps_q2")
                gemm(ps[:, 0:T], wqt2, oc, lambda ic: x1pT[:, ic, :], WC)
                nc.vector.tensor_copy(q2T[:, oc, :], ps[:, 0:T])

        wkt2 = load_wT(ca_w, "wt8A", dram['ca_wk'], W, W, "ca_wk_t",
                       dtype=F8, scale16=True, ev='act')
        wot2 = load_wT(ca_w, "wtA", dram['ca_wo'], W, W, "ca_wo_t",
                       ev='act', dtype=BF16)

        ocaT = ca.tile([128, WC, T], BF16, tag="ocaT", name="ocaT")
        x2T = persist.tile([128, WC, T], BF16, tag="xper2", name="x2T")
        with tc.tile_pool(name="ca_acc", bufs=1) as cacc, \
             tc.tile_pool(name="ca_jc", bufs=1) as cjc:
            for b in range(B_loc):
                acc = cacc.tile([HD + 1, NH, NQ], F32, tag="acc",
                                name="acc_ca")
                for g in range(NJC // G):
                    k2Ts, vexts = [], []
                    for jg in range(G):
                        jc = g * G + jg
                        tok0 = b * S + jc * JC
                        m_tm = []
                        for sj in range(JSC):
                            mt = cjc.tile([128, W], F32R, tag="m_tm", bufs=4,
                                          name="m_tm")
                            nc.sync.dma_start(
                                mt[:],
                                m_flat[tok0 + sj * 128:
                                       tok0 + (sj + 1) * 128, :]
                                .bitcast(F32R))
                            m_tm.append(mt)
                        mT = cjc.tile([128, WC, JC], F8, tag="mT",
                                      bufs=2, name="mT")
                        for fp_ in range(WC // 2):
                            transpose_group(
                                mT[:, fp_ * 2:fp_ * 2 + 2, 0:JC],
                                [m_tm[sj][:, fc * 128:(fc + 1) * 128]
                                 for fc in (fp_ * 2, fp_ * 2 + 1)
                                 for sj in range(JSC)], n2=JC)
                        k2T = cjc.tile([128, WC, JC], BF16, tag="k2T",
                                       bufs=2 * G, name="k2T")
                        for op_ in range(WC // 2):
                            ps = mm_ps.tile([128, 2, JC], F32, tag="mm",
                                            name="ps_k2")
                            for sub in range(2):
                                oc = op_ * 2 + sub
                                for icp in range(WC // 2):
                                    nc.tensor.matmul(
                                        ps[:, sub, :],
                                        wkt2[:, 2 * icp:2 * icp + 2,
                                             oc * 128:(oc + 1) * 128],
                                        mT[:, 2 * icp:2 * icp + 2, :],
                                        start=(icp == 0),
                                        stop=(icp == WC // 2 - 1),
                                        perf_mode=DR)
                            nc.vector.tensor_copy(
                                k2T[:, op_ * 2:op_ * 2 + 2, :],
                                ps[:, :, 0:JC])
                        vext = cjc.tile([128, JSC, NH, HD + 1], F8,
                                        tag="vext", bufs=2 * G, name="vext_ca")
                        for sj in range(JSC):
                            for op2 in range(W // 512):
                                ps = mm_ps.tile([128, 2, 256], F32, tag="mm",
                                                name="ps_v2")
                                for half in range(2):
                                    oh = op2 * 2 + half
                                    for icp in range(WC // 2):
                                        nc.tensor.matmul(
                                            ps[:, half, :],
                                            mT[:, 2 * icp:2 * icp + 2,
                                               sj * 128:(sj + 1) * 128],
                                            wvt2[:, 2 * icp:2 * icp + 2,
                                                 oh * 256:(oh + 1) * 256],
                                            start=(icp == 0),
                                            stop=(icp == WC // 2 - 1),
                                            perf_mode=DR)
                                nh0 = op2 * (512 // HD)
                                nc.vector.tensor_copy(
                                    vext[:, sj, nh0:nh0 + 512 // HD, 0:HD],
                                    ps[:, :, :].rearrange(
                                        "p a (h d) -> p (a h) d", d=HD))
                            nc.gpsimd.memset(vext[:, sj, :, HD], W8SCALE)
                        k2Ts.append(k2T)
                        vexts.append(vext)
                    for hp in range(NH // 2):
                        ps_o = pv_ps.tile([HD + 1, 2, NQ], F32, tag="pv",
                                          name="ps_pv2")
                        for jg in range(G):
                            es = [head_scores(b, 2 * hp + sub, k2Ts[jg],
                                              q2T, SCALE / W8SCALE)
                                  for sub in range(2)]
                            for sub in range(2):
                                nc.tensor.matmul(
                                    ps_o[0:HD + 1, sub, :],
                                    vexts[jg][:, :, 2 * hp + sub, :],
                                    es[sub][:, :, :],
                                    start=(jg == 0), stop=(jg == G - 1),
                                    perf_mode=DR)
                        if g == 0:
                            nc.vector.tensor_copy(
                                acc[0:HD + 1, 2 * hp:2 * hp + 2, :],
                                ps_o[0:HD + 1, :, :])
                        else:
                            nc.vector.tensor_add(
                                acc[0:HD + 1, 2 * hp:2 * hp + 2, :],
                                acc[0:HD + 1, 2 * hp:2 * hp + 2, :],
                                ps_o[0:HD + 1, :, :])
                        if g == NJC // G - 1:
                            for sub in range(2):
                                normalize_head(
                                    2 * hp + sub, acc[:, 2 * hp + sub, :],
                                    ocaT[:, :, b * NQ:(b + 1) * NQ])
                x2pre = cacc.tile([128, WC, NQ], F32R, tag="x2pre", bufs=2,
                                  name="x2pre")
                for oc in range(WC):
                    ps = mm_ps.tile([128, NQ], F32, tag="mm", name="ps_o2")
                    gemm(ps[:, 0:NQ], wot2, oc,
                         lambda ic: ocaT[:, ic, b * NQ:(b + 1) * NQ], WC)
                    nc.vector.tensor_add(x2pre[:, oc, :], ps[:, 0:NQ],
                                         x1T[:, oc, b * NQ:(b + 1) * NQ])
                layernorm(lambda ic: x2pre[:, ic, :], WC, NQ,
                          cols['ln2_g'], cols['ln2_b'],
                          lambda ic: x2T[:, ic, b * NQ:(b + 1) * NQ],
                          xm_pool=(b == 0))

    # ================= FFN =================
    with tc.tile_pool(name="ffn", bufs=1) as ffn:
        hT = ffn.tile([128, MC, T], BF16, tag="hT", name="hT")
        for oc in range(MC):
            w1t = ffn.tile([128, WC, 128], BF16, tag="w1t", bufs=2, name="w1t")
            wr = ffn.tile([128, W], F32R, tag="w1row", bufs=3, name="wr1")
            nc.sync.dma_start(
                wr[:], dram['ffn_w1'][oc * 128:(oc + 1) * 128, :].bitcast(F32R))
            for half in range(W // 512):
                transpose_group(
                    w1t[:, half * 4:(half + 1) * 4, :],
                    [wr[:, half * 512 + k * 128:half * 512 + (k + 1) * 128]
                     for k in range(4)])
            ps = mm_ps.tile([128, T], F32, tag="mm", name="ps_h")
            for ic in range(WC):
                nc.tensor.matmul(ps[:, 0:T], w1t[:, ic, :], x2T[:, ic, :],
                                 start=(ic == 0), stop=(ic == WC - 1))
            nc.scalar.activation(hT[:, oc, :], ps[:, 0:T], AF.Relu,
                                 bias=b1_col[:, oc:oc + 1])
        x3pre = ffn.tile([128, WC, T], F32R, tag="x3pre", name="x3pre")
        for oc in range(WC):
            w2t = ffn.tile([128, MC, 128], BF16, tag="w2t", bufs=2, name="w2t")
            PIECE = 1024 if MLP % 1024 == 0 else 512
            for piece in range(MLP // PIECE):
                wr = ffn.tile([128, PIECE], F32R, tag="w2row", bufs=3,
                              name="wr2", padded_shape=[128, 1024])
                nc.sync.dma_start(
                    wr[:], dram['ffn_w2'][oc * 128:(oc + 1) * 128,
                                          piece * PIECE:(piece + 1) * PIECE]
                    .bitcast(F32R))
                for hh in range(PIECE // 512):
                    half = piece * (PIECE // 512) + hh
                    transpose_group(
                        w2t[:, half * 4:(half + 1) * 4, :],
                        [wr[:, hh * 512 + k * 128:hh * 512 + (k + 1) * 128]
                         for k in range(4)])
            ps = mm_ps.tile([128, T], F32, tag="mm", name="ps_f")
            for ic in range(MC):
                nc.tensor.matmul(ps[:, 0:T], w2t[:, ic, :], hT[:, ic, :],
                                 start=(ic == 0), stop=(ic == MC - 1))
            tmp = scratch.tile([128, T], F32, tag="sq", name="f_tmp")
            nc.scalar.activation(tmp[:, 0:T], ps[:, 0:T], AF.Identity,
                                 bias=cols['ffn_b2'][:, oc:oc + 1])
            nc.vector.tensor_add(x3pre[:, oc, :], tmp[:, 0:T], x2T[:, oc, :])
        x3T = ffn.tile([128, WC, T], F32, tag="x3T", name="x3T")
        for b in range(B_loc):
            n0 = b * NQ
            layernorm(lambda ic: x3pre[:, ic, n0:n0 + NQ], WC, NQ,
                      cols['ln3_g'], cols['ln3_b'],
                      lambda ic: x3T[:, ic, n0:n0 + NQ], xm_pool=False)
            for tcx in range(b * NQC, (b + 1) * NQC):
                o_tm = ffn.tile([128, W], F32, tag="o_tm", bufs=2, name="o_tm")
                for g in range(WC // 4):
                    transpose_group(
                        o_tm[:, g * 512:(g + 1) * 512],
                        [x3T[:, g * 4 + k, tcx * 128:(tcx + 1) * 128]
                         for k in range(4)], exact=True)
                nc.sync.dma_start(out_flat[tcx * 128:(tcx + 1) * 128, :],
                                  o_tm[:])

    return out_d


_PROGRAM_CACHE = {}


def _get_program(B_loc, NQ, S, W, NH, MLP, JC=256, repeat=1):
    key = (B_loc, NQ, S, W, NH, MLP, JC, repeat)
    if key not in _PROGRAM_CACHE:
        nc = bacc.Bacc("TRN2", target_bir_lowering=False, debug=False)
        with tile.TileContext(nc) as tc, \
             nc.allow_low_precision(reason="fp32r/fp8 matmul pipeline"):
            for r in range(repeat):
                with ExitStack() as ctx:
                    build_decoder(nc, tc, ctx, B_loc, NQ, S, W, NH, MLP, JC,
                                  suffix=("" if r == 0 else f"_r{r}"))
        nc.compile()
        _PROGRAM_CACHE[key] = nc
    return _PROGRAM_CACHE[key]


def kernel(**inputs):
    B, NQ, W = inputs['query'].shape
    S = inputs['enc_mem'].shape[1]
    MLP = inputs['ffn_w1'].shape[0]
    NH = 16
    assert B % N_CORES == 0
    B_loc = B // N_CORES

    nc = _get_program(B_loc, NQ, S, W, NH, MLP)

    shard_names = {'query', 'enc_mem', 'out_pos_enc'}
    in_maps = []
    for c in range(N_CORES):
        m = {}
        for k, v in inputs.items():
            v = np.ascontiguousarray(np.asarray(v, dtype=np.float32))
            if k in shard_names:
                m[k] = np.ascontiguousarray(v[c * B_loc:(c + 1) * B_loc])
            else:
                m[k] = v
        in_maps.append(m)

    res = run_bass_kernel_spmd(nc, in_maps, list(range(N_CORES)))
    return np.concatenate([res.results[c]["out"] for c in range(N_CORES)],
                          axis=0)
